# revision 10
# baseline (speedup 1.0000x reference)
"""Trainium2 Bass kernel for nn_Attention_50173807952647.

GQA attention block: qkv projections + partial interleaved RoPE + softmax
attention + output projection, fp32 inputs/outputs.

Sharding: 8 cores; core d owns kv-head d and query heads {2d, 2d+1} for all
4 batches (head/tensor parallel per the GQA grouping). Each core computes a
partial output (its heads' contribution through Wo); host sums partials + bias.

v2 design notes (vs baseline):
  - x^T is pre-transposed and cast to bf16 on the HOST (layout prep, like the
    host-side cos/sin tables) -> no on-chip x transposes, no PSUM->SBUF x^T
    casts. Cores DMA x^T slabs straight to SBUF.
  - All PE matmul operands are bf16 (same 1 cyc/row as f32r but half the
    SBUF/DMA bytes); PSUM accumulation stays f32.
  - attnV merged to one matmul per k-tile (N=1024 covering both heads).
  - ACT engine does exp ONLY; the softmax epilogue copies run on Pool/DVE.
  - Output partials stored as bf16 (host sums in f32).
"""

import sys

import numpy as np

HEADS = 16
KV_HEADS = 8
DIM_HEAD = 64
ROT_DIM = 32
SCALE = DIM_HEAD ** -0.5
B, N, DIM = 4, 2048, 1024
N_CORES = 8
T = B * N  # 8192 tokens
CHUNK = 512  # projection chunk (tokens)
QB = 512  # attention query block

_BUILT = {}


def _ensure_path():
    for p in ("/opt/trn_rl_repo",):
        if p not in sys.path:
            sys.path.insert(0, p)


def _rope_tables():
    """cos/sin tables [128, N] for the transposed [hd, t] layout.

    Row r (hd index within a core's 128 q-rows): head-local d = r % 64.
    d < ROT_DIM: cos(t * inv_freq[d//2]); sin with rotate-half sign folded
    (-sin on even d, +sin on odd d). Elsewhere cos=1, sin=0 so a single
    full-width mul+add applies RoPE only where it belongs.
    """
    inv_freq = 1.0 / (10000.0 ** (np.arange(0, ROT_DIM, 2, dtype=np.float64) / ROT_DIM))
    t = np.arange(N, dtype=np.float64)
    freqs = t[None, :] * inv_freq[:, None]  # [16, N]
    cos = np.ones((128, N), dtype=np.float64)
    sin = np.zeros((128, N), dtype=np.float64)
    for r in range(128):
        d = r % 64
        if d < ROT_DIM:
            f = freqs[d // 2]
            cos[r] = np.cos(f)
            sin[r] = (-1.0 if d % 2 == 0 else 1.0) * np.sin(f)
    return cos.astype(np.float32), sin.astype(np.float32)


def _build():
    if "nc" in _BUILT:
        return _BUILT["nc"]
    _ensure_path()
    import concourse.bass as bass  # noqa: F401
    import concourse.mybir as mybir
    import concourse.tile as tile
    from concourse import bacc
    from concourse.masks import make_identity

    dt = mybir.dt
    f32, bf16 = dt.float32, dt.bfloat16
    AF = mybir.ActivationFunctionType
    OP = mybir.AluOpType

    nc = bacc.Bacc("TRN2", target_bir_lowering=False, debug=False)

    xt_in = nc.dram_tensor("xt", [DIM, T], bf16, kind="ExternalInput").ap()
    wq_in = nc.dram_tensor("wq", [DIM, 128], bf16, kind="ExternalInput").ap()
    wkv_in = nc.dram_tensor("wkv", [DIM, 128], bf16, kind="ExternalInput").ap()
    wo_in = nc.dram_tensor("wo", [128, DIM], bf16, kind="ExternalInput").ap()
    cos_in = nc.dram_tensor("cos_t", [128, N], f32, kind="ExternalInput").ap()
    sin_in = nc.dram_tensor("sin_t", [128, N], f32, kind="ExternalInput").ap()
    out_d = nc.dram_tensor("out", [T, DIM], bf16, kind="ExternalOutput").ap()

    NCH = N // CHUNK  # chunks per batch
    NQB = N // QB  # q blocks per batch
    NKT = N // 128  # key tiles per batch
    pair_mask = []
    for i in range(16):
        pair_mask += [2 * i + 1, 2 * i]

    with tile.TileContext(nc) as tc:
        with (
            tc.tile_pool(name="const", bufs=1) as constp,
            tc.tile_pool(name="perbatch", bufs=2) as batchp,
            tc.tile_pool(name="xt", bufs=2) as xtp,
            tc.tile_pool(name="rope", bufs=6) as ropep,
            tc.tile_pool(name="sm", bufs=2) as smp,
            tc.tile_pool(name="exp", bufs=6) as expp,
            tc.tile_pool(name="osb", bufs=4) as osbp,
            tc.tile_pool(name="outsb", bufs=3) as outsbp,
            tc.tile_pool(name="psA", bufs=2, space="PSUM") as psA,
            tc.tile_pool(name="psB", bufs=1, space="PSUM") as psB,
            tc.tile_pool(name="psC", bufs=2, space="PSUM") as psC,
        ):
            ident = constp.tile([128, 128], f32)
            make_identity(nc, ident[:])
            wq_sb = constp.tile([128, 8 * 128], bf16, tag="wq")
            wkv_sb = constp.tile([128, 8 * 128], bf16, tag="wkv")
            for et in range(8):
                nc.scalar.dma_start(wq_sb[:, et * 128:(et + 1) * 128],
                                    wq_in[et * 128:(et + 1) * 128, :])
                nc.scalar.dma_start(wkv_sb[:, et * 128:(et + 1) * 128],
                                    wkv_in[et * 128:(et + 1) * 128, :])
            wo_sb = constp.tile([128, DIM], bf16, tag="wo")
            nc.scalar.dma_start(wo_sb[:], wo_in[:])
            cos_sb = constp.tile([128, N], f32, tag="cos")
            sin_sb = constp.tile([128, N], f32, tag="sin")
            nc.scalar.dma_start(cos_sb[:], cos_in[:])
            nc.scalar.dma_start(sin_sb[:], sin_in[:])

            def proj_chunk(b, c, tiles):
                """Generator: project chunk c of batch b into tiles.

                DMAs x^T slabs, runs q/kv projections, RoPE epilogue, and the
                v transpose fixup. Yields between PE instructions so the
                caller can interleave it into an attention k-tile stream.
                """
                qT, kT0, kT1, v_sb = tiles
                cs = slice(c * CHUNK, (c + 1) * CHUNK)
                xTt = xtp.tile([128, 8 * CHUNK], bf16, tag="xT")
                t0 = b * N + c * CHUNK
                for et in range(8):
                    nc.sync.dma_start(
                        xTt[:, et * CHUNK:(et + 1) * CHUNK],
                        xt_in[et * 128:(et + 1) * 128, t0:t0 + CHUNK])
                yield
                qps = psC.tile([128, 512], f32, tag="ps_small")
                for et in range(8):
                    nc.tensor.matmul(qps[:],
                                     wq_sb[:, et * 128:(et + 1) * 128],
                                     xTt[:, et * CHUNK:(et + 1) * CHUNK],
                                     start=(et == 0), stop=(et == 7))
                    if et % 2 == 1:
                        yield
                kvps = psC.tile([128, 512], f32, tag="ps_small")
                for et in range(8):
                    nc.tensor.matmul(kvps[:],
                                     wkv_sb[:, et * 128:(et + 1) * 128],
                                     xTt[:, et * CHUNK:(et + 1) * CHUNK],
                                     start=(et == 0), stop=(et == 7))
                    if et % 2 == 1:
                        yield
                # rope epilogue: q
                shq = ropep.tile([128, CHUNK], f32, tag="rope")
                nc.vector.stream_shuffle(shq[:], qps[:], pair_mask)
                t1q = ropep.tile([128, CHUNK], f32, tag="rope")
                nc.vector.tensor_tensor(t1q[:], qps[:], cos_sb[:, cs], op=OP.mult)
                t2q = ropep.tile([128, CHUNK], f32, tag="rope")
                nc.vector.tensor_tensor(t2q[:], shq[:], sin_sb[:, cs], op=OP.mult)
                nc.vector.tensor_tensor(qT[:, cs], t1q[:], t2q[:], op=OP.add)
                yield
                # rope epilogue: k -> kT0 rows 0:64 (mul written in place;
                # rotary rows fixed up with an in-place add)
                shk = ropep.tile([32, CHUNK], f32, tag="rope")
                nc.vector.stream_shuffle(shk[:], kvps[0:32, :], pair_mask)
                nc.vector.tensor_tensor(kT0[0:64, cs], kvps[0:64, :],
                                        cos_sb[0:64, cs], op=OP.mult)
                t2k = ropep.tile([32, CHUNK], f32, tag="rope")
                nc.vector.tensor_tensor(t2k[:], shk[:], sin_sb[0:32, cs], op=OP.mult)
                nc.vector.tensor_tensor(kT0[0:32, cs], kT0[0:32, cs], t2k[:],
                                        op=OP.add)
                nc.sync.dma_start(kT1[64:128, cs], kT0[0:64, cs])
                yield
                # v fixup: transpose v^T [64, tok] -> v natural [tok, 64]
                vts = ropep.tile([64, CHUNK], f32, tag="rope")
                nc.vector.tensor_copy(vts[:], kvps[64:128, :])
                yield
                vtp = psC.tile([128, 512], f32, tag="ps_small")
                for st in range(4):
                    nc.tensor.transpose(vtp[:, st * 128: st * 128 + 64],
                                        vts[:, st * 128:(st + 1) * 128],
                                        ident[0:64, 0:64])
                    yield
                vsrc = vtp[:].rearrange("p (st x) -> p st x", x=128)[:, :, 0:64]
                vdst = v_sb[:, c * 260:(c + 1) * 260].rearrange(
                    "p (kt x) -> p kt x", x=65)[:, :, 0:64]
                nc.vector.tensor_copy(vdst, vsrc)
                yield

            def attn_core(b, qb, tiles, fillers=()):
                qT, kT0, kT1, v_sb = tiles
                qs = slice(qb * QB, (qb + 1) * QB)
                ops_t = psB.tile([65, 1024], f32, tag="ps_o")
                es = []
                for kt in range(NKT):
                    sps = psA.tile([128, 1024], f32, tag="ps_big")
                    nc.tensor.matmul(sps[:, 0:512],
                                     kT0[:, kt * 128:(kt + 1) * 128],
                                     qT[:, qs], start=True, stop=True)
                    nc.tensor.matmul(sps[:, 512:1024],
                                     kT1[:, kt * 128:(kt + 1) * 128],
                                     qT[:, qs], start=True, stop=True)
                    e_sb = expp.tile([128, 1024], bf16, tag="e")
                    nc.scalar.activation(e_sb[:], sps[:], AF.Exp, scale=SCALE)
                    es.append(e_sb)
                    for f in fillers:
                        next(f, None)
                    # software-pipeline: attnV lags scores by 2 k-tiles so the
                    # exp dependency is already complete (no PE sem-wait stall)
                    if kt >= 2:
                        j = kt - 2
                        nc.tensor.matmul(ops_t[:, 0:512],
                                         v_sb[:, j * 65: j * 65 + 65],
                                         es[j][:, 0:512],
                                         start=(j == 0), stop=False)
                        nc.tensor.matmul(ops_t[:, 512:1024],
                                         v_sb[:, j * 65: j * 65 + 65],
                                         es[j][:, 512:1024],
                                         start=(j == 0), stop=False)
                for j in (NKT - 2, NKT - 1):
                    nc.tensor.matmul(ops_t[:, 0:512],
                                     v_sb[:, j * 65: j * 65 + 65],
                                     es[j][:, 0:512], start=False,
                                     stop=(j == NKT - 1))
                    nc.tensor.matmul(ops_t[:, 512:1024],
                                     v_sb[:, j * 65: j * 65 + 65],
                                     es[j][:, 512:1024], start=False,
                                     stop=(j == NKT - 1))
                ouden = smp.tile([65, 1024], f32, tag="ouden")
                nc.vector.tensor_copy(ouden[:], ops_t[:])
                return ouden

            def epilogue_vec(ouden):
                """Softmax normalize for a finished q-block (DVE/Pool only)."""
                den = smp.tile([1, 1024], f32, tag="den")
                nc.vector.tensor_copy(den[:], ouden[64:65, :])
                rec = smp.tile([1, 1024], f32, tag="rq")
                nc.vector.reciprocal_approx_fast(rec[:], den[:])
                rb = smp.tile([64, 1024], f32, tag="rb")
                nc.gpsimd.partition_broadcast(rb[:], rec[:])
                oT = osbp.tile([128, QB], bf16, tag="o")
                nc.vector.tensor_tensor(oT[0:64, :], ouden[0:64, 0:512],
                                        rb[0:64, 0:512], op=OP.mult)
                o1 = osbp.tile([64, QB], bf16, tag="o1")
                nc.vector.tensor_tensor(o1[:], ouden[0:64, 512:1024],
                                        rb[0:64, 512:1024], op=OP.mult)
                nc.sync.dma_start(oT[64:128, :], o1[:])
                return oT

            def outproj_gen(b, qb, oT):
                """Generator: out-projection matmuls, interleavable into the
                next q-block's k-tile stream (lagged so oT is ready)."""
                for _ in range(4):
                    yield
                for ts in range(4):
                    for eh in range(2):
                        po = psC.tile([128, 512], f32, tag="ps_small")
                        nc.tensor.matmul(po[:],
                                         oT[:, ts * 128:(ts + 1) * 128],
                                         wo_sb[:, eh * 512:(eh + 1) * 512],
                                         start=True, stop=True)
                        ob = outsbp.tile([128, 512], bf16, tag="ob")
                        nc.vector.tensor_copy(ob[:], po[:])
                        r0 = b * N + qb * QB + ts * 128
                        nc.sync.dma_start(
                            out_d[r0:r0 + 128, eh * 512:(eh + 1) * 512], ob[:])
                        yield

            def batch_tiles(b):
                qT = batchp.tile([128, N], bf16, tag="qT")
                kT0 = batchp.tile([128, N], bf16, tag="kT0")
                kT1 = batchp.tile([128, N], bf16, tag="kT1")
                v_sb = batchp.tile([128, NKT * 65], bf16, tag="v")
                nc.vector.memset(kT0[64:128, :], 0.0)
                nc.vector.memset(kT1[0:64, :], 0.0)
                ones = v_sb[:].rearrange("p (kt c) -> p kt c", c=65)[:, :, 64:65]
                nc.vector.memset(ones, 1.0)
                return (qT, kT0, kT1, v_sb)

            # software-pipelined emission: the PE-idle window while ACT works
            # through each q-block's exp stream is filled with the next
            # batch's projection chunk.
            tiles = batch_tiles(0)
            for c in range(NCH):
                for _ in proj_chunk(0, c, tiles):
                    pass
            prev = None  # (b, qb, ouden) pending out-proj, one iteration behind
            for b in range(B):
                nxt = batch_tiles(b + 1) if b + 1 < B else None
                for i in range(NQB):
                    fillers = []
                    if prev is not None:
                        oT = epilogue_vec(prev[2])
                        fillers.append(outproj_gen(prev[0], prev[1], oT))
                    if nxt is not None:
                        fillers.append(proj_chunk(b + 1, i, nxt))
                    o = attn_core(b, i, tiles, fillers=fillers)
                    for g in fillers:
                        for _ in g:
                            pass
                    prev = (b, i, o)
                if nxt is not None:
                    tiles = nxt
            oT = epilogue_vec(prev[2])
            for _ in outproj_gen(prev[0], prev[1], oT):
                pass

    nc.compile()
    _BUILT["nc"] = nc
    return nc


def _make_in_maps(x, Wq, Wk, Wv, Wo):
    import ml_dtypes
    bf16 = ml_dtypes.bfloat16
    cos_t, sin_t = _rope_tables()
    xt = np.ascontiguousarray(
        np.asarray(x, np.float32).reshape(T, DIM).T.astype(bf16))
    in_maps = []
    for d in range(N_CORES):
        wq_d = np.ascontiguousarray(Wq[:, d * 128:(d + 1) * 128].astype(bf16))
        wk_d = Wk[:, d * 64:(d + 1) * 64]
        wv_d = Wv[:, d * 64:(d + 1) * 64]
        wkv_d = np.ascontiguousarray(
            np.concatenate([wk_d, wv_d], axis=1).astype(bf16))
        wo_d = np.ascontiguousarray(Wo[d * 128:(d + 1) * 128, :].astype(bf16))
        in_maps.append({
            "xt": xt, "wq": wq_d, "wkv": wkv_d, "wo": wo_d,
            "cos_t": cos_t, "sin_t": sin_t,
        })
    return in_maps


def _run(in_maps, trace=False, trace_kwargs=None):
    _ensure_path()
    from concourse.bass_utils import run_bass_kernel_spmd
    nc = _build()
    return run_bass_kernel_spmd(nc, in_maps, list(range(N_CORES)), trace=trace,
                                **(trace_kwargs or {}))


def kernel(x, Wq, Wk, Wv, Wo, bo):
    x = np.asarray(x, dtype=np.float32)
    in_maps = _make_in_maps(x, np.asarray(Wq, np.float32), np.asarray(Wk, np.float32),
                            np.asarray(Wv, np.float32), np.asarray(Wo, np.float32))
    res = _run(in_maps)
    acc = np.zeros((T, DIM), dtype=np.float32)
    for d in range(N_CORES):
        acc += np.asarray(res.results[d]["out"], dtype=np.float32)
    acc += np.asarray(bo, np.float32)[None, :]
    return acc.reshape(B, N, DIM)


# revision 13
# speedup vs baseline: 1.0087x; 1.0087x over previous
"""Trainium2 Bass kernel for nn_Attention_50173807952647.

GQA attention block: qkv projections + partial interleaved RoPE + softmax
attention + output projection, fp32 inputs/outputs.

Sharding: 8 cores; core d owns kv-head d and query heads {2d, 2d+1} for all
4 batches (head/tensor parallel per the GQA grouping). Each core computes a
partial output (its heads' contribution through Wo); host sums partials + bias.

v2 design notes (vs baseline):
  - x^T is pre-transposed and cast to bf16 on the HOST (layout prep, like the
    host-side cos/sin tables) -> no on-chip x transposes, no PSUM->SBUF x^T
    casts. Cores DMA x^T slabs straight to SBUF.
  - All PE matmul operands are bf16 (same 1 cyc/row as f32r but half the
    SBUF/DMA bytes); PSUM accumulation stays f32.
  - attnV merged to one matmul per k-tile (N=1024 covering both heads).
  - ACT engine does exp ONLY; the softmax epilogue copies run on Pool/DVE.
  - Output partials stored as bf16 (host sums in f32).
"""

import sys

import numpy as np

HEADS = 16
KV_HEADS = 8
DIM_HEAD = 64
ROT_DIM = 32
SCALE = DIM_HEAD ** -0.5
B, N, DIM = 4, 2048, 1024
N_CORES = 8
T = B * N  # 8192 tokens
CHUNK = 512  # projection chunk (tokens)
QB = 512  # attention query block

_BUILT = {}


def _ensure_path():
    for p in ("/opt/trn_rl_repo",):
        if p not in sys.path:
            sys.path.insert(0, p)


def _rope_tables():
    """cos/sin tables [128, N] for the transposed [hd, t] layout.

    Row r (hd index within a core's 128 q-rows): head-local d = r % 64.
    d < ROT_DIM: cos(t * inv_freq[d//2]); sin with rotate-half sign folded
    (-sin on even d, +sin on odd d). Elsewhere cos=1, sin=0 so a single
    full-width mul+add applies RoPE only where it belongs.
    """
    inv_freq = 1.0 / (10000.0 ** (np.arange(0, ROT_DIM, 2, dtype=np.float64) / ROT_DIM))
    t = np.arange(N, dtype=np.float64)
    freqs = t[None, :] * inv_freq[:, None]  # [16, N]
    cos = np.ones((128, N), dtype=np.float64)
    sin = np.zeros((128, N), dtype=np.float64)
    for r in range(128):
        d = r % 64
        if d < ROT_DIM:
            f = freqs[d // 2]
            cos[r] = np.cos(f)
            sin[r] = (-1.0 if d % 2 == 0 else 1.0) * np.sin(f)
    return cos.astype(np.float32), sin.astype(np.float32)


def _build():
    if "nc" in _BUILT:
        return _BUILT["nc"]
    _ensure_path()
    import concourse.bass as bass  # noqa: F401
    import concourse.mybir as mybir
    import concourse.tile as tile
    from concourse import bacc
    from concourse.masks import make_identity

    dt = mybir.dt
    f32, bf16 = dt.float32, dt.bfloat16
    AF = mybir.ActivationFunctionType
    OP = mybir.AluOpType

    nc = bacc.Bacc("TRN2", target_bir_lowering=False, debug=False)

    xt_in = nc.dram_tensor("xt", [DIM, T], bf16, kind="ExternalInput").ap()
    wq_in = nc.dram_tensor("wq", [DIM, 128], bf16, kind="ExternalInput").ap()
    wkv_in = nc.dram_tensor("wkv", [DIM, 128], bf16, kind="ExternalInput").ap()
    wo_in = nc.dram_tensor("wo", [128, DIM], bf16, kind="ExternalInput").ap()
    cos_in = nc.dram_tensor("cos_t", [128, N], f32, kind="ExternalInput").ap()
    sin_in = nc.dram_tensor("sin_t", [128, N], f32, kind="ExternalInput").ap()
    out_d = nc.dram_tensor("out", [T, DIM], bf16, kind="ExternalOutput").ap()

    NCH = N // CHUNK  # chunks per batch
    NQB = N // QB  # q blocks per batch
    NKT = N // 128  # key tiles per batch
    pair_mask = []
    for i in range(16):
        pair_mask += [2 * i + 1, 2 * i]

    with tile.TileContext(nc) as tc:
        with (
            tc.tile_pool(name="const", bufs=1) as constp,
            tc.tile_pool(name="perbatch", bufs=2) as batchp,
            tc.tile_pool(name="xt", bufs=2) as xtp,
            tc.tile_pool(name="rope", bufs=6) as ropep,
            tc.tile_pool(name="sm", bufs=2) as smp,
            tc.tile_pool(name="exp", bufs=6) as expp,
            tc.tile_pool(name="osb", bufs=4) as osbp,
            tc.tile_pool(name="outsb", bufs=3) as outsbp,
            tc.tile_pool(name="psA", bufs=2, space="PSUM") as psA,
            tc.tile_pool(name="psB", bufs=1, space="PSUM") as psB,
            tc.tile_pool(name="psC", bufs=2, space="PSUM") as psC,
        ):
            ident = constp.tile([128, 128], f32)
            make_identity(nc, ident[:])
            wq_sb = constp.tile([128, 8 * 128], bf16, tag="wq")
            wkv_sb = constp.tile([128, 8 * 128], bf16, tag="wkv")
            for et in range(8):
                nc.scalar.dma_start(wq_sb[:, et * 128:(et + 1) * 128],
                                    wq_in[et * 128:(et + 1) * 128, :])
                nc.scalar.dma_start(wkv_sb[:, et * 128:(et + 1) * 128],
                                    wkv_in[et * 128:(et + 1) * 128, :])
            wo_sb = constp.tile([128, DIM], bf16, tag="wo")
            nc.scalar.dma_start(wo_sb[:], wo_in[:])
            cos_sb = constp.tile([128, N], f32, tag="cos")
            sin_sb = constp.tile([128, N], f32, tag="sin")
            nc.scalar.dma_start(cos_sb[:], cos_in[:])
            nc.scalar.dma_start(sin_sb[:], sin_in[:])

            def proj_chunk(b, c, tiles):
                """Generator: project chunk c of batch b into tiles.

                DMAs x^T slabs, runs q/kv projections, RoPE epilogue, and the
                v transpose fixup. Yields between PE instructions so the
                caller can interleave it into an attention k-tile stream.
                """
                qT, kT, v_sb = tiles
                cs = slice(c * CHUNK, (c + 1) * CHUNK)
                xTt = xtp.tile([128, 8 * CHUNK], bf16, tag="xT")
                t0 = b * N + c * CHUNK
                for et in range(8):
                    nc.sync.dma_start(
                        xTt[:, et * CHUNK:(et + 1) * CHUNK],
                        xt_in[et * 128:(et + 1) * 128, t0:t0 + CHUNK])
                yield
                qps = psC.tile([128, 512], f32, tag="ps_small")
                for et in range(8):
                    nc.tensor.matmul(qps[:],
                                     wq_sb[:, et * 128:(et + 1) * 128],
                                     xTt[:, et * CHUNK:(et + 1) * CHUNK],
                                     start=(et == 0), stop=(et == 7))
                    if et % 2 == 1:
                        yield
                kvps = psC.tile([128, 512], f32, tag="ps_small")
                for et in range(8):
                    nc.tensor.matmul(kvps[:],
                                     wkv_sb[:, et * 128:(et + 1) * 128],
                                     xTt[:, et * CHUNK:(et + 1) * CHUNK],
                                     start=(et == 0), stop=(et == 7))
                    if et % 2 == 1:
                        yield
                # rope epilogue: q
                shq = ropep.tile([128, CHUNK], f32, tag="rope")
                nc.vector.stream_shuffle(shq[:], qps[:], pair_mask)
                t1q = ropep.tile([128, CHUNK], f32, tag="rope")
                nc.vector.tensor_tensor(t1q[:], qps[:], cos_sb[:, cs], op=OP.mult)
                t2q = ropep.tile([128, CHUNK], f32, tag="rope")
                nc.vector.tensor_tensor(t2q[:], shq[:], sin_sb[:, cs], op=OP.mult)
                nc.vector.tensor_tensor(qT[:, cs], t1q[:], t2q[:], op=OP.add)
                yield
                # rope epilogue: k -> kT0 rows 0:64 (mul written in place;
                # rotary rows fixed up with an in-place add)
                shk = ropep.tile([32, CHUNK], f32, tag="rope")
                nc.vector.stream_shuffle(shk[:], kvps[0:32, :], pair_mask)
                nc.vector.tensor_tensor(kT[0:64, cs], kvps[0:64, :],
                                        cos_sb[0:64, cs], op=OP.mult)
                t2k = ropep.tile([32, CHUNK], f32, tag="rope")
                nc.vector.tensor_tensor(t2k[:], shk[:], sin_sb[0:32, cs], op=OP.mult)
                nc.vector.tensor_tensor(kT[0:32, cs], kT[0:32, cs], t2k[:],
                                        op=OP.add)
                nc.sync.dma_start(kT[64:128, cs], kT[0:64, cs])
                yield
                # v fixup: transpose v^T [64, tok] -> v natural [tok, 64]
                vts = ropep.tile([64, CHUNK], f32, tag="rope")
                nc.vector.tensor_copy(vts[:], kvps[64:128, :])
                yield
                vtp = psC.tile([128, 512], f32, tag="ps_small")
                for st in range(4):
                    nc.tensor.transpose(vtp[:, st * 128: st * 128 + 64],
                                        vts[:, st * 128:(st + 1) * 128],
                                        ident[0:64, 0:64])
                    yield
                vsrc = vtp[:].rearrange("p (st x) -> p st x", x=128)[:, :, 0:64]
                vdst = v_sb[:, c * 260:(c + 1) * 260].rearrange(
                    "p (kt x) -> p kt x", x=65)[:, :, 0:64]
                nc.vector.tensor_copy(vdst, vsrc)
                yield

            def attn_core(b, qb, tiles, fillers=()):
                qT, kT, v_sb = tiles
                qs = slice(qb * QB, (qb + 1) * QB)
                ops_t = psB.tile([65, 1024], f32, tag="ps_o")
                es = []
                for kt in range(NKT):
                    sps = psA.tile([128, 1024], f32, tag="ps_big")
                    nc.tensor.matmul(sps[:, 0:512],
                                     kT[0:64, kt * 128:(kt + 1) * 128],
                                     qT[0:64, qs], start=True, stop=True,
                                     tile_position=(0, 0))
                    nc.tensor.matmul(sps[:, 512:1024],
                                     kT[64:128, kt * 128:(kt + 1) * 128],
                                     qT[64:128, qs], start=True, stop=True,
                                     tile_position=(64, 0))
                    e_sb = expp.tile([128, 1024], bf16, tag="e")
                    nc.scalar.activation(e_sb[:], sps[:], AF.Exp, scale=SCALE)
                    es.append(e_sb)
                    for f in fillers:
                        next(f, None)
                    # software-pipeline: attnV lags scores by 2 k-tiles so the
                    # exp dependency is already complete (no PE sem-wait stall)
                    if kt >= 2:
                        j = kt - 2
                        nc.tensor.matmul(ops_t[:, 0:512],
                                         v_sb[:, j * 65: j * 65 + 65],
                                         es[j][:, 0:512],
                                         start=(j == 0), stop=False)
                        nc.tensor.matmul(ops_t[:, 512:1024],
                                         v_sb[:, j * 65: j * 65 + 65],
                                         es[j][:, 512:1024],
                                         start=(j == 0), stop=False)
                for j in (NKT - 2, NKT - 1):
                    nc.tensor.matmul(ops_t[:, 0:512],
                                     v_sb[:, j * 65: j * 65 + 65],
                                     es[j][:, 0:512], start=False,
                                     stop=(j == NKT - 1))
                    nc.tensor.matmul(ops_t[:, 512:1024],
                                     v_sb[:, j * 65: j * 65 + 65],
                                     es[j][:, 512:1024], start=False,
                                     stop=(j == NKT - 1))
                ouden = smp.tile([65, 1024], f32, tag="ouden")
                nc.vector.tensor_copy(ouden[:], ops_t[:])
                return ouden

            def epilogue_vec(ouden):
                """Softmax normalize for a finished q-block (DVE/Pool only)."""
                den = smp.tile([1, 1024], f32, tag="den")
                nc.vector.tensor_copy(den[:], ouden[64:65, :])
                rec = smp.tile([1, 1024], f32, tag="rq")
                nc.vector.reciprocal_approx_fast(rec[:], den[:])
                rb = smp.tile([64, 1024], f32, tag="rb")
                nc.gpsimd.partition_broadcast(rb[:], rec[:])
                oT = osbp.tile([128, QB], bf16, tag="o")
                nc.vector.tensor_tensor(oT[0:64, :], ouden[0:64, 0:512],
                                        rb[0:64, 0:512], op=OP.mult)
                o1 = osbp.tile([64, QB], bf16, tag="o1")
                nc.vector.tensor_tensor(o1[:], ouden[0:64, 512:1024],
                                        rb[0:64, 512:1024], op=OP.mult)
                nc.sync.dma_start(oT[64:128, :], o1[:])
                return oT

            def outproj_gen(b, qb, oT):
                """Generator: out-projection matmuls, interleavable into the
                next q-block's k-tile stream (lagged so oT is ready)."""
                for _ in range(4):
                    yield
                for ts in range(4):
                    for eh in range(2):
                        po = psC.tile([128, 512], f32, tag="ps_small")
                        nc.tensor.matmul(po[:],
                                         oT[:, ts * 128:(ts + 1) * 128],
                                         wo_sb[:, eh * 512:(eh + 1) * 512],
                                         start=True, stop=True)
                        ob = outsbp.tile([128, 512], bf16, tag="ob")
                        nc.vector.tensor_copy(ob[:], po[:])
                        r0 = b * N + qb * QB + ts * 128
                        nc.sync.dma_start(
                            out_d[r0:r0 + 128, eh * 512:(eh + 1) * 512], ob[:])
                        yield

            def batch_tiles(b):
                qT = batchp.tile([128, N], bf16, tag="qT")
                kT = batchp.tile([128, N], bf16, tag="kT")
                v_sb = batchp.tile([128, NKT * 65], bf16, tag="v")
                ones = v_sb[:].rearrange("p (kt c) -> p kt c", c=65)[:, :, 64:65]
                nc.vector.memset(ones, 1.0)
                return (qT, kT, v_sb)

            # software-pipelined emission: the PE-idle window while ACT works
            # through each q-block's exp stream is filled with the next
            # batch's projection chunk.
            tiles = batch_tiles(0)
            for c in range(NCH):
                for _ in proj_chunk(0, c, tiles):
                    pass
            prev = None  # (b, qb, ouden) pending out-proj, one iteration behind
            for b in range(B):
                nxt = batch_tiles(b + 1) if b + 1 < B else None
                for i in range(NQB):
                    fillers = []
                    if prev is not None:
                        oT = epilogue_vec(prev[2])
                        fillers.append(outproj_gen(prev[0], prev[1], oT))
                    if nxt is not None:
                        fillers.append(proj_chunk(b + 1, i, nxt))
                    o = attn_core(b, i, tiles, fillers=fillers)
                    for g in fillers:
                        for _ in g:
                            pass
                    prev = (b, i, o)
                if nxt is not None:
                    tiles = nxt
            oT = epilogue_vec(prev[2])
            for _ in outproj_gen(prev[0], prev[1], oT):
                pass

    nc.compile()
    _BUILT["nc"] = nc
    return nc


def _make_in_maps(x, Wq, Wk, Wv, Wo):
    import ml_dtypes
    bf16 = ml_dtypes.bfloat16
    cos_t, sin_t = _rope_tables()
    xt = np.ascontiguousarray(
        np.asarray(x, np.float32).reshape(T, DIM).T.astype(bf16))
    in_maps = []
    for d in range(N_CORES):
        wq_d = np.ascontiguousarray(Wq[:, d * 128:(d + 1) * 128].astype(bf16))
        wk_d = Wk[:, d * 64:(d + 1) * 64]
        wv_d = Wv[:, d * 64:(d + 1) * 64]
        wkv_d = np.ascontiguousarray(
            np.concatenate([wk_d, wv_d], axis=1).astype(bf16))
        wo_d = np.ascontiguousarray(Wo[d * 128:(d + 1) * 128, :].astype(bf16))
        in_maps.append({
            "xt": xt, "wq": wq_d, "wkv": wkv_d, "wo": wo_d,
            "cos_t": cos_t, "sin_t": sin_t,
        })
    return in_maps


def _run(in_maps, trace=False, trace_kwargs=None):
    _ensure_path()
    from concourse.bass_utils import run_bass_kernel_spmd
    nc = _build()
    return run_bass_kernel_spmd(nc, in_maps, list(range(N_CORES)), trace=trace,
                                **(trace_kwargs or {}))


def kernel(x, Wq, Wk, Wv, Wo, bo):
    x = np.asarray(x, dtype=np.float32)
    in_maps = _make_in_maps(x, np.asarray(Wq, np.float32), np.asarray(Wk, np.float32),
                            np.asarray(Wv, np.float32), np.asarray(Wo, np.float32))
    res = _run(in_maps)
    acc = np.zeros((T, DIM), dtype=np.float32)
    for d in range(N_CORES):
        acc += np.asarray(res.results[d]["out"], dtype=np.float32)
    acc += np.asarray(bo, np.float32)[None, :]
    return acc.reshape(B, N, DIM)
